# revision 1
# baseline (speedup 1.0000x reference)
"""Bass/Tile TRN2 kernel for BasicAttention.

att = softmax(tanh(hidden @ W_h.T + p_att_feats) @ W_alpha + mask) @ att_feats

Shapes: B=64, N=2048, H=1024, A=512. Data-parallel over batch across 8
NeuronCores (8 batches per core); weights replicated; no collectives.

Layout: region index n maps to (partition p, column c) as n = p*16 + c so
every p_att/att_feats DMA is a long contiguous per-partition read.

Per-core dataflow (memory-bound: ~102MB HBM reads/core; ~427 GB/s solo /
~340 GB/s when all 8 cores stream against their shared HBM stacks):
  host: pass W_h.T / hidden.T (bf16) / masks / one-hot selector pre-tiled
        so each is ONE dma, plus a pre-broadcast bf16 W_alpha (layout-only
        transforms + weight-only bf16 cast).
  setup: the first p_att loads are issued before any const DMA (Sync issues
         in order); w_h = hidden @ W_h.T (PE, bf16) -> per-batch partition
         broadcast of w_h rows via one-hot PE matmuls (no DRAM round-trip);
         PSUM drained by ACT so DVE stays clear for the streaming adds.
  per batch b (software-pipelined, p_att phase leads att_feats phase):
    p_att stream [128,8,512]: DVE add (w_h bcast) -> ACT tanh (bf16)
      -> DVE scalar_tensor_tensor vs W_alpha (accum) -> scores[128,16]
    scores: + mask, ACT exp (accum rowsum, f32r out), PE total-sum,
      DVE reciprocal
    att_feats stream [128,4,1024] f32r: PE matmuls (attn col stationary)
      accumulating att[1,1024] in PSUM.
    epilogue OFF the load-bearing queues: ACT drains PSUM with the 1/sum
      scale folded in (scale=inv); output DMA via GpSimd (SWDGE) so its
      data-dependent wait cannot head-of-line-block the Sync DMA queue.
"""

import numpy as np

B, N, H, A = 64, 2048, 1024, 512
NCORES = 8
BLOC = B // NCORES  # batches per core

P = 128
NT = N // P            # 16 n-columns per partition
PATT_SUP = 8           # columns per p_att supertile (2 DMAs per batch)
AF_SUP = 4             # columns per att_feats supertile (4 DMAs per batch)
HC = H // P            # 8 contraction chunks for the w_h matmul
# last batch streams att_feats in finer chunks so the post-DMA tail is
# short; its final column is split into lo/hi half-loads so the last
# accumulation + drain + output write chain is half-length
AF_SPLIT_LAST = (4, 4, 4, 2, 1)

_NC_CACHE = {}


def _free_bcast(bass_mod, ap, repeat):
    """[P, F] AP -> [P, repeat, F] AP with 0-stride middle dim."""
    return bass_mod.AP(
        tensor=ap.tensor,
        offset=ap.offset,
        ap=[ap.ap[0], [0, repeat], *ap.ap[1:]],
    )


def _build_nc():
    import concourse.bass as bass
    import concourse.mybir as mybir
    import concourse.tile as tile
    from concourse import bacc

    dt = mybir.dt
    f32, f32r, bf16 = dt.float32, dt.float32r, dt.bfloat16
    AF = mybir.ActivationFunctionType
    OP = mybir.AluOpType

    nc = bacc.Bacc("TRN2", target_bir_lowering=False, debug=False,
                   num_devices=NCORES)

    # host pre-tiles weights so each const load is a single DMA
    hsT = nc.dram_tensor("hidden_T", [P, HC, BLOC], bf16,
                         kind="ExternalInput").ap()
    af = nc.dram_tensor("att_feats", [BLOC, N, H], f32r, kind="ExternalInput").ap()
    pa = nc.dram_tensor("p_att_feats", [BLOC, N, A], f32, kind="ExternalInput").ap()
    am = nc.dram_tensor("att_masks", [P, BLOC, NT], f32, kind="ExternalInput").ap()
    whT = nc.dram_tensor("W_hT", [P, HC, A], bf16, kind="ExternalInput").ap()
    e8d = nc.dram_tensor("e8_sel", [BLOC, BLOC * P], f32r,
                         kind="ExternalInput").ap()
    wa1 = nc.dram_tensor("W_alpha_r", [1, A], f32, kind="ExternalInput").ap()
    out = nc.dram_tensor("att_out", [BLOC, H], f32, kind="ExternalOutput").ap()

    with tile.TileContext(nc) as tc:
        with (
            tc.tile_pool(name="consts", bufs=1) as consts,
            tc.tile_pool(name="patt", bufs=4) as patt_pool,
            tc.tile_pool(name="alpha", bufs=3) as alpha_pool,
            tc.tile_pool(name="afp", bufs=4) as af_pool,
            tc.tile_pool(name="small", bufs=4) as small,
            tc.tile_pool(name="arow", bufs=2) as arow_pool,
            tc.tile_pool(name="psmisc", bufs=2, space="PSUM") as psmisc,
            tc.tile_pool(name="psatt", bufs=6, space="PSUM") as psatt,
        ):
            # ---------------- setup ----------------
            # issue the first two big streaming loads before any const DMA so
            # the HBM stream starts as early as possible (Sync issues in order)
            pa_r = [pa[b, :, :].rearrange("(p c) a -> p c a", c=NT)
                    for b in range(BLOC)]
            pre0 = []
            for st in range(NT // PATT_SUP):
                pt = patt_pool.tile([P, PATT_SUP, A], f32, tag="patt",
                                    name=f"patt0_{st}")
                nc.sync.dma_start(
                    out=pt,
                    in_=pa_r[0][:, st * PATT_SUP:(st + 1) * PATT_SUP, :],
                )
                pre0.append(pt)

            whT_sb = consts.tile([P, HC, A], bf16)
            nc.sync.dma_start(out=whT_sb, in_=whT)
            hidT_sb = consts.tile([P, HC, BLOC], bf16)
            nc.sync.dma_start(out=hidT_sb, in_=hsT)
            wa_sb1 = consts.tile([1, A], f32)
            nc.sync.dma_start(out=wa_sb1, in_=wa1)
            masks_all = consts.tile([P, BLOC, NT], f32)
            nc.sync.dma_start(out=masks_all, in_=am)

            ones_col = consts.tile([P, 1], f32)
            nc.vector.memset(ones_col, 1.0)
            ones_row = consts.tile([1, P], f32)
            nc.vector.memset(ones_row, 1.0)

            # broadcast W_alpha across partitions on-chip (exact: x*1)
            wa_ps = psmisc.tile([P, A], f32, tag="mm", name="wa_ps")
            nc.tensor.matmul(wa_ps, lhsT=ones_row, rhs=wa_sb1,
                             start=True, stop=True)
            wa_bf = consts.tile([P, A], bf16)
            nc.scalar.copy(wa_bf, wa_ps)
            # one-hot selector rows: e8[k, b*P+p] = (k == b), host-built
            e8 = consts.tile([BLOC, BLOC * P], f32r)
            nc.sync.dma_start(out=e8, in_=e8d)

            # w_h = hidden @ W_h.T : [8, 512] (f32r, 1 cyc/row)
            wh_ps = psmisc.tile([BLOC, A], f32, tag="mm")
            for hc in range(HC):
                nc.tensor.matmul(wh_ps, lhsT=hidT_sb[:, hc, :],
                                 rhs=whT_sb[:, hc, :],
                                 start=(hc == 0), stop=(hc == HC - 1))
            whall_sb = consts.tile([BLOC, A], f32r)
            nc.scalar.copy(whall_sb, wh_ps)

            # broadcast each w_h row across all 128 partitions on-chip:
            # whb[b][p, a] = sum_k e8[k, b*P+p] * whall[k, a] = w_h[b, a]
            whb = []
            for b in range(BLOC):
                bc_ps = psmisc.tile([P, A], f32, tag="mm", name=f"bcps{b}")
                nc.tensor.matmul(bc_ps, lhsT=e8[:, b * P:(b + 1) * P],
                                 rhs=whall_sb, start=True, stop=True)
                t = consts.tile([P, A], f32, name=f"whb{b}", tag=f"whb{b}")
                nc.scalar.copy(t, bc_ps)
                whb.append(t)

            # ---------------- main loop (software-pipelined) ----------------
            # n = p*NT + c everywhere below.
            af_r = [af[b, :, :].rearrange("(p c) h -> p c h", c=NT)
                    for b in range(BLOC)]

            def patt_phase(b):
                scores = small.tile([P, NT], f32, tag="scores",
                                    name=f"scores{b}")
                for st in range(NT // PATT_SUP):  # 2 supertiles
                    if b == 0:
                        pt = pre0[st]
                    else:
                        pt = patt_pool.tile([P, PATT_SUP, A], f32, tag="patt",
                                            name=f"patt{b}_{st}")
                        nc.sync.dma_start(
                            out=pt,
                            in_=pa_r[b][:, st * PATT_SUP:(st + 1) * PATT_SUP, :],
                        )
                    whb_b = _free_bcast(bass, whb[b][:, :], PATT_SUP)
                    nc.vector.tensor_tensor(out=pt, in0=pt, in1=whb_b, op=OP.add)
                    ab = alpha_pool.tile([P, PATT_SUP, A], bf16, tag="alpha",
                                         name=f"alpha{b}_{st}")
                    nc.scalar.activation(ab, pt, AF.Tanh)
                    for c in range(PATT_SUP):
                        col = st * PATT_SUP + c
                        # out = (ab * 1) * wa ; accum_out = row-sum -> scores
                        nc.vector.scalar_tensor_tensor(
                            out=ab[:, c, :], in0=ab[:, c, :], scalar=1.0,
                            in1=wa_bf, op0=OP.mult, op1=OP.mult,
                            accum_out=scores[:, col:col + 1],
                        )

                nc.vector.tensor_tensor(out=scores, in0=scores,
                                        in1=masks_all[:, b, :], op=OP.add)

                expt = small.tile([P, NT], f32r, tag="expt", name=f"expt{b}")
                rowsum = small.tile([P, 1], f32, tag="rowsum",
                                    name=f"rowsum{b}")
                nc.scalar.activation(expt, scores, AF.Exp, accum_out=rowsum)

                sum_ps = psmisc.tile([1, 1], f32, tag="mm", name=f"sum_ps{b}")
                nc.tensor.matmul(sum_ps, lhsT=rowsum, rhs=ones_col,
                                 start=True, stop=True)
                inv = small.tile([1, 1], f32, tag="inv", name=f"inv{b}")
                nc.vector.reciprocal(inv, sum_ps)
                return expt, inv

            def af_phase(b, expt, inv):
                att_lo = psatt.tile([1, A], f32, tag="att", name=f"attlo{b}")
                att_hi = psatt.tile([1, A], f32, tag="att", name=f"atthi{b}")
                split = AF_SPLIT_LAST if b == BLOC - 1 else \
                    (AF_SUP,) * (NT // AF_SUP)
                t = 0
                for st2, w in enumerate(split):
                    aft = af_pool.tile([P, w, H], f32r, tag="af",
                                       name=f"af{b}_{st2}")
                    nc.sync.dma_start(out=aft, in_=af_r[b][:, t:t + w, :])
                    for c in range(w):
                        lhs = expt[:, t:t + 1]
                        nc.tensor.matmul(att_lo, lhsT=lhs,
                                         rhs=aft[:, c, 0:A],
                                         start=(t == 0), stop=(t == NT - 1))
                        nc.tensor.matmul(att_hi, lhsT=lhs,
                                         rhs=aft[:, c, A:H],
                                         start=(t == 0), stop=(t == NT - 1))
                        t += 1
                if b == BLOC - 1:
                    # final column as lo/hi half-loads: the lo-half chain
                    # (matmul, drain, output write) completes while the
                    # hi half is still in flight
                    lhs = expt[:, t:t + 1]
                    for half, (h0, h1) in enumerate(((0, A), (A, H))):
                        afh = af_pool.tile([P, 1, A], f32r, tag="af",
                                           name=f"af{b}_h{half}")
                        nc.sync.dma_start(
                            out=afh, in_=af_r[b][:, t:t + 1, h0:h1])
                        nc.tensor.matmul(att_lo if half == 0 else att_hi,
                                         lhsT=lhs, rhs=afh[:, 0, :],
                                         start=False, stop=True)

                # epilogue on ACT + GpSimd only: drain PSUM with the 1/sum
                # scale folded in, then a SWDGE output write
                att_row = arow_pool.tile([1, H], f32, tag="attrow",
                                         name=f"attrow{b}")
                nc.scalar.activation(att_row[:, 0:A], att_lo, AF.Copy,
                                     scale=inv[0:1, 0:1])
                if b == BLOC - 1:
                    # tail: the lo half drains and writes out while the hi
                    # half-column load/matmul is still in flight; hi drains
                    # on the (empty) DVE queue, writes on the (empty) Sync
                    nc.sync.dma_start(out=out[b:b + 1, 0:A],
                                      in_=att_row[:, 0:A])
                    nc.vector.tensor_scalar_mul(att_row[:, A:H], att_hi, inv)
                    nc.sync.dma_start(out=out[b:b + 1, A:H],
                                      in_=att_row[:, A:H])
                else:
                    nc.scalar.activation(att_row[:, A:H], att_hi, AF.Copy,
                                         scale=inv[0:1, 0:1])
                    nc.gpsimd.dma_start(out=out[b:b + 1, :], in_=att_row)

            state = {}
            for b in range(BLOC):
                state[b] = patt_phase(b)
                if b >= 1:
                    af_phase(b - 1, *state.pop(b - 1))
            af_phase(BLOC - 1, *state.pop(BLOC - 1))

    nc.compile()
    return nc


def _get_nc():
    if "nc" not in _NC_CACHE:
        _NC_CACHE["nc"] = _build_nc()
    return _NC_CACHE["nc"]


def kernel(hidden_states, att_feats, p_att_feats, att_masks, W_h, W_alpha):
    import ml_dtypes
    from concourse.bass_utils import run_bass_kernel_spmd

    nc = _get_nc()
    hidden_states = np.ascontiguousarray(hidden_states, dtype=np.float32)
    att_feats = np.ascontiguousarray(att_feats, dtype=np.float32)
    p_att_feats = np.ascontiguousarray(p_att_feats, dtype=np.float32)
    att_masks = np.ascontiguousarray(att_masks, dtype=np.float32)
    W_h = np.ascontiguousarray(W_h, dtype=np.float32)
    W_alpha = np.asarray(W_alpha, dtype=np.float32).reshape(1, A)

    # [H, A] -> [P, HC, A] so the whole thing is one DMA; bf16 halves traffic
    whT = np.ascontiguousarray(
        W_h.T.reshape(HC, P, A).transpose(1, 0, 2)).astype(ml_dtypes.bfloat16)
    e8 = np.ascontiguousarray(np.repeat(np.eye(BLOC, dtype=np.float32),
                                        P, axis=1))

    in_maps = []
    for i in range(NCORES):
        s = slice(i * BLOC, (i + 1) * BLOC)
        # [H, BLOC] -> [P, HC, BLOC]
        hT = np.ascontiguousarray(
            hidden_states[s].T.reshape(HC, P, BLOC).transpose(1, 0, 2)
        ).astype(ml_dtypes.bfloat16)
        # [BLOC, N] -> [P, BLOC, NT] with n = p*NT + c
        amr = np.ascontiguousarray(
            att_masks[s].reshape(BLOC, P, NT).transpose(1, 0, 2))
        in_maps.append({
            "hidden_T": hT,
            "att_feats": att_feats[s],
            "p_att_feats": p_att_feats[s],
            "att_masks": amr,
            "W_hT": whT,
            "W_alpha_r": W_alpha,
            "e8_sel": e8,
        })

    global _LAST_IN_MAPS
    _LAST_IN_MAPS = in_maps
    res = run_bass_kernel_spmd(nc, in_maps, core_ids=list(range(NCORES)))
    return np.concatenate(
        [res.results[i]["att_out"] for i in range(NCORES)], axis=0
    ).astype(np.float32)


_LAST_IN_MAPS = None



# revision 2
# speedup vs baseline: 1.7404x; 1.7404x over previous
"""Bass/Tile TRN2 kernel for BasicAttention (v2: low-precision streams +
PE-based scores).

att = softmax(tanh(hidden @ W_h.T + p_att_feats) @ W_alpha + mask) @ att_feats

Shapes: B=64, N=2048, H=1024, A=512. Data-parallel over batch across 8
NeuronCores (8 batches per core); weights replicated; no collectives.

v2 design (baseline was all-f32 streams, DVE-based scores, 259-310us):
  * host casts the two big streams: att_feats -> bf16, p_att_feats ->
    fp8e4m3 (sim rel err 7.2e-3 vs 2e-2 budget). HBM traffic/core drops
    102MB -> ~43MB.
  * p_att is host-TRANSPOSED to [b, j, a=128p, n]: the A axis lives on
    partitions, so w_h becomes a per-partition scalar and the whole
    w_h-add + tanh fuses into ONE ACT op (bias=w_hT column). The DVE
    add (70us) disappears.
  * scores = alpha.T @ W_alpha move from DVE stt (87us) to PE matmuls
    (lhsT=W_alphaT column, rhs=alpha chunk, accumulate 4 a-chunks into
    [1,512] PSUM rows): ~28us warm on an otherwise half-idle PE.
  * score rows drain via DVE to a [1,2048] row, then one tiny DMA on the
    ACT HWDGE queue transposes it to the region-partition layout
    [128,16] (n = p*16 + c) needed as att-matmul lhsT. Mask add (DVE,
    FD=16) + exp (ACT, FD=16, accum rowsum) happen on the nice layout.
  * att matmuls, softmax denominator (PE ones-sum), 1/sum-scaled PSUM
    drain and SWDGE output write are as in the baseline.

Engine budget/core @ ~16.5us/batch steady state: DMA ~111-130us (floor),
ACT ~85us, PE ~90us warm, DVE ~25us.

Layouts: region n maps to (partition p, column c) as n = p*16 + c for
att_feats/masks/scores; a maps to (j, p) as a = j*128 + p for p_att/W_h.
"""

import numpy as np

B, N, H, A = 64, 2048, 1024, 512
NCORES = 8
BLOC = B // NCORES  # batches per core

P = 128
NT = N // P            # 16 n-columns per partition (att layout)
AJ = A // P            # 4 a-chunks (p_att layout)
NN = N // 512          # 4 score chunks of 512
HC = H // P            # 8 contraction chunks for the w_h matmul
AF_SUP = 4             # columns per att_feats supertile (4 DMAs per batch)
# last batch streams att_feats in finer chunks so the post-DMA tail is
# short; its final column is split into lo/hi half-loads
AF_SPLIT_LAST = (4, 4, 4, 2, 1)

# p_att stream dtype: fp8e4m3 (sim rel err 7.2e-3). Flip to bf16 if
# device error disagrees with the numpy sim.
PA_FP8 = True

_NC_CACHE = {}


def _build_nc():
    import concourse.bass as bass
    import concourse.mybir as mybir
    import concourse.tile as tile
    from concourse import bacc

    dt = mybir.dt
    f32, bf16 = dt.float32, dt.bfloat16
    pa_dt = dt.float8e4 if PA_FP8 else bf16
    AF = mybir.ActivationFunctionType
    OP = mybir.AluOpType

    nc = bacc.Bacc("TRN2", target_bir_lowering=False, debug=False,
                   num_devices=NCORES)

    # host pre-tiles everything so each const load is a single DMA
    paT = nc.dram_tensor("p_att_T", [BLOC, AJ, P, N], pa_dt,
                         kind="ExternalInput").ap()
    af = nc.dram_tensor("att_feats", [BLOC, N, H], bf16,
                        kind="ExternalInput").ap()
    whTj = nc.dram_tensor("W_hTj", [AJ, P, HC, P], bf16,
                          kind="ExternalInput").ap()
    hsT = nc.dram_tensor("hidden_T", [P, HC, BLOC], bf16,
                         kind="ExternalInput").ap()
    am = nc.dram_tensor("att_masks", [P, BLOC, NT], f32,
                        kind="ExternalInput").ap()
    waT = nc.dram_tensor("W_alphaT", [P, AJ], bf16, kind="ExternalInput").ap()
    out = nc.dram_tensor("att_out", [BLOC, H], f32, kind="ExternalOutput").ap()

    with tile.TileContext(nc) as tc:
        with (
            tc.tile_pool(name="consts", bufs=1) as consts,
            tc.tile_pool(name="patt", bufs=6) as patt_pool,
            tc.tile_pool(name="alpha", bufs=5) as alpha_pool,
            tc.tile_pool(name="afp", bufs=4) as af_pool,
            tc.tile_pool(name="small", bufs=3) as small,
            tc.tile_pool(name="arow", bufs=2) as arow_pool,
            tc.tile_pool(name="psc", bufs=4, space="PSUM") as psc_pool,
            tc.tile_pool(name="psatt", bufs=2, space="PSUM") as psatt,
            tc.tile_pool(name="psmisc", bufs=2, space="PSUM") as psmisc,
        ):
            # ---------------- setup ----------------
            # interleave W_h chunk loads with the first batch's p_att
            # chunks so tanh(0, j) can start as early as possible: the
            # Sync queue issues in order.
            whT_sb = []
            pre0 = []
            hidT_sb = consts.tile([P, HC, BLOC], bf16)
            for j in range(AJ):
                wt = consts.tile([P, HC, P], bf16, name=f"whT{j}")
                nc.sync.dma_start(out=wt, in_=whTj[j, :, :, :])
                whT_sb.append(wt)
                if j == 0:
                    nc.sync.dma_start(out=hidT_sb, in_=hsT)
                pt = patt_pool.tile([P, N], pa_dt, tag="patt",
                                    name=f"patt0_{j}")
                nc.sync.dma_start(out=pt, in_=paT[0, j, :, :])
                pre0.append(pt)
            waT_sb = consts.tile([P, AJ], bf16)
            nc.sync.dma_start(out=waT_sb, in_=waT)
            masks_all = consts.tile([P, BLOC, NT], f32)
            nc.sync.dma_start(out=masks_all, in_=am)

            ones_col = consts.tile([P, 1], f32)
            nc.vector.memset(ones_col, 1.0)

            # w_hT[p, j, b] = w_h[b, j*128+p] = sum_h W_h[j*128+p, h] *
            # hidden[b, h]: stationary W_h chunks put the A axis on the
            # OUTPUT partitions, giving the transposed bias directly.
            whbias = consts.tile([P, AJ, BLOC], f32)
            for j in range(AJ):
                wh_ps = psc_pool.tile([P, BLOC], f32, tag="sc",
                                      name=f"whps{j}")
                for hc in range(HC):
                    nc.tensor.matmul(wh_ps, lhsT=whT_sb[j][:, hc, :],
                                     rhs=hidT_sb[:, hc, :],
                                     start=(hc == 0), stop=(hc == HC - 1))
                nc.scalar.copy(whbias[:, j, :], wh_ps)

            # ---------------- main loop (software-pipelined) ----------------
            af_r = [af[b, :, :].rearrange("(p c) h -> p c h", c=NT)
                    for b in range(BLOC)]

            def patt_phase(b):
                # stream p_att chunks; fused add+tanh per a-chunk
                alphas = []
                for j in range(AJ):
                    if b == 0:
                        pt = pre0[j]
                    else:
                        pt = patt_pool.tile([P, N], pa_dt, tag="patt",
                                            name=f"patt{b}_{j}")
                        nc.sync.dma_start(out=pt, in_=paT[b, j, :, :])
                    ab = alpha_pool.tile([P, N], bf16, tag="alpha",
                                         name=f"alpha{b}_{j}")
                    nc.scalar.activation(ab, pt, AF.Tanh,
                                         bias=whbias[:, j, b:b + 1])
                    alphas.append(ab)

                # scores on PE: row nn holds scores for n in
                # [512nn, 512nn+512); accumulate the 4 a-chunks
                row4 = small.tile([1, N], f32, tag="row", name=f"row{b}")
                for nn in range(NN):
                    scp = psc_pool.tile([1, 512], f32, tag="sc",
                                        name=f"sc{b}_{nn}")
                    for j in range(AJ):
                        nc.tensor.matmul(
                            scp, lhsT=waT_sb[:, j:j + 1],
                            rhs=alphas[j][:, nn * 512:(nn + 1) * 512],
                            start=(j == 0), stop=(j == AJ - 1))
                    nc.vector.tensor_copy(row4[:, nn * 512:(nn + 1) * 512],
                                          scp)

                # one tiny DMA transposes [1, 2048] -> [128, 16]
                # (n = p*16 + c); rides the ACT HWDGE queue so the Sync
                # streaming queue never sees a data-dependent wait.
                scT = small.tile([P, NT], f32, tag="scT", name=f"scT{b}")
                nc.scalar.dma_start(out=scT, in_=row4)

                nc.vector.tensor_tensor(out=scT, in0=scT,
                                        in1=masks_all[:, b, :], op=OP.add)
                expt = small.tile([P, NT], bf16, tag="expt", name=f"expt{b}")
                rowsum = small.tile([P, 1], f32, tag="rowsum",
                                    name=f"rowsum{b}")
                nc.scalar.activation(expt, scT, AF.Exp, accum_out=rowsum)
                return expt, rowsum

            def af_phase(b, expt, rowsum):
                sum_ps = psmisc.tile([1, 1], f32, tag="mm", name=f"sum{b}")
                nc.tensor.matmul(sum_ps, lhsT=rowsum, rhs=ones_col,
                                 start=True, stop=True)
                inv = small.tile([1, 1], f32, tag="inv", name=f"inv{b}")
                nc.vector.reciprocal(inv, sum_ps)

                att_lo = psatt.tile([1, A], f32, tag="att", name=f"attlo{b}")
                att_hi = psatt.tile([1, A], f32, tag="att", name=f"atthi{b}")
                split = AF_SPLIT_LAST if b == BLOC - 1 else \
                    (AF_SUP,) * (NT // AF_SUP)
                t = 0
                for st2, w in enumerate(split):
                    aft = af_pool.tile([P, w, H], bf16, tag="af",
                                       name=f"af{b}_{st2}")
                    nc.sync.dma_start(out=aft, in_=af_r[b][:, t:t + w, :])
                    for c in range(w):
                        lhs = expt[:, t:t + 1]
                        nc.tensor.matmul(att_lo, lhsT=lhs,
                                         rhs=aft[:, c, 0:A],
                                         start=(t == 0), stop=(t == NT - 1))
                        nc.tensor.matmul(att_hi, lhsT=lhs,
                                         rhs=aft[:, c, A:H],
                                         start=(t == 0), stop=(t == NT - 1))
                        t += 1
                if b == BLOC - 1:
                    # final column as lo/hi half-loads so the last
                    # accumulate + drain + output chain is half-length
                    lhs = expt[:, t:t + 1]
                    for half, (h0, h1) in enumerate(((0, A), (A, H))):
                        afh = af_pool.tile([P, 1, A], bf16, tag="af",
                                           name=f"af{b}_h{half}")
                        nc.sync.dma_start(
                            out=afh, in_=af_r[b][:, t:t + 1, h0:h1])
                        nc.tensor.matmul(att_lo if half == 0 else att_hi,
                                         lhsT=lhs, rhs=afh[:, 0, :],
                                         start=False, stop=True)

                # epilogue: drain PSUM with the 1/sum scale folded in,
                # then a SWDGE output write
                att_row = arow_pool.tile([1, H], f32, tag="attrow",
                                         name=f"attrow{b}")
                nc.scalar.activation(att_row[:, 0:A], att_lo, AF.Copy,
                                     scale=inv[0:1, 0:1])
                if b == BLOC - 1:
                    # tail: lo half drains and writes out while the hi
                    # half-column load/matmul is still in flight
                    nc.sync.dma_start(out=out[b:b + 1, 0:A],
                                      in_=att_row[:, 0:A])
                    nc.vector.tensor_scalar_mul(att_row[:, A:H], att_hi, inv)
                    nc.sync.dma_start(out=out[b:b + 1, A:H],
                                      in_=att_row[:, A:H])
                else:
                    nc.scalar.activation(att_row[:, A:H], att_hi, AF.Copy,
                                         scale=inv[0:1, 0:1])
                    nc.gpsimd.dma_start(out=out[b:b + 1, :], in_=att_row)

            state = {}
            for b in range(BLOC):
                state[b] = patt_phase(b)
                if b >= 1:
                    af_phase(b - 1, *state.pop(b - 1))
            af_phase(BLOC - 1, *state.pop(BLOC - 1))

    nc.compile()
    return nc


def _get_nc():
    if "nc" not in _NC_CACHE:
        _NC_CACHE["nc"] = _build_nc()
    return _NC_CACHE["nc"]


def kernel(hidden_states, att_feats, p_att_feats, att_masks, W_h, W_alpha):
    import ml_dtypes
    from concourse.bass_utils import run_bass_kernel_spmd

    nc = _get_nc()
    pa_np = ml_dtypes.float8_e4m3fn if PA_FP8 else ml_dtypes.bfloat16
    hidden_states = np.ascontiguousarray(hidden_states, dtype=np.float32)
    att_feats = np.ascontiguousarray(att_feats, dtype=np.float32)
    p_att_feats = np.ascontiguousarray(p_att_feats, dtype=np.float32)
    att_masks = np.ascontiguousarray(att_masks, dtype=np.float32)
    W_h = np.ascontiguousarray(W_h, dtype=np.float32)
    W_alpha = np.asarray(W_alpha, dtype=np.float32)

    # W_hTj[j, p, hc, m] = W_h[j*128+m, hc*128+p]
    whTj = np.ascontiguousarray(
        W_h.reshape(AJ, P, HC, P).transpose(0, 3, 2, 1)
    ).astype(ml_dtypes.bfloat16)
    waT = np.ascontiguousarray(
        W_alpha.reshape(AJ, P).T).astype(ml_dtypes.bfloat16)

    in_maps = []
    for i in range(NCORES):
        s = slice(i * BLOC, (i + 1) * BLOC)
        # [H, BLOC] -> [P, HC, BLOC]
        hT = np.ascontiguousarray(
            hidden_states[s].T.reshape(HC, P, BLOC).transpose(1, 0, 2)
        ).astype(ml_dtypes.bfloat16)
        # [BLOC, N] -> [P, BLOC, NT] with n = p*16 + c
        amr = np.ascontiguousarray(
            att_masks[s].reshape(BLOC, P, NT).transpose(1, 0, 2))
        # [BLOC, N, A] -> [BLOC, AJ, P, N] with a = j*128 + p
        paT = p_att_feats[s].transpose(0, 2, 1).reshape(
            BLOC, AJ, P, N).astype(pa_np)
        in_maps.append({
            "p_att_T": paT,
            "att_feats": att_feats[s].astype(ml_dtypes.bfloat16),
            "hidden_T": hT,
            "att_masks": amr,
            "W_hTj": whTj,
            "W_alphaT": waT,
        })

    global _LAST_IN_MAPS
    _LAST_IN_MAPS = in_maps
    res = run_bass_kernel_spmd(nc, in_maps, core_ids=list(range(NCORES)))
    return np.concatenate(
        [res.results[i]["att_out"] for i in range(NCORES)], axis=0
    ).astype(np.float32)


_LAST_IN_MAPS = None


# revision 3
# speedup vs baseline: 1.8339x; 1.0537x over previous
"""Bass/Tile TRN2 kernel for BasicAttention (v3: low-precision streams,
PE-based scores, full double-buffer prefetch).

att = softmax(tanh(hidden @ W_h.T + p_att_feats) @ W_alpha + mask) @ att_feats

Shapes: B=64, N=2048, H=1024, A=512. Data-parallel over batch across 8
NeuronCores (8 batches per core); weights replicated; no collectives.

Design (baseline all-f32 was 259-310us; v2 = 178us; v3 targets ~120us):
  * host casts the streams: att_feats -> bf16, p_att_feats -> fp8e4m3
    (device rel err 7.2e-3 vs 2e-2 budget). HBM: 102MB -> ~43MB/core.
  * p_att host-transposed to [b, j, a=128p, n]: A on partitions, so the
    w_h add is a per-partition scalar and add+tanh fuse into ONE ACT op
    (bias=w_hT column). No DVE add.
  * scores = alpha.T @ W_alpha on PE (lhsT=W_alphaT column, rhs=alpha
    chunk, 4 a-chunks accumulate into [1,512] PSUM rows).
  * score rows drain via DVE to a [1,2048] row; one tiny ACT-queue DMA
    transposes to region-partition layout [128,16] (n = p*16+c); mask
    add (DVE) + exp (ACT, accum rowsum) there; denominator via PE
    ones-matmul; att drains on DVE with 1/sum folded; SWDGE output.
  * both streams prefetch one full batch ahead on SEPARATE queues
    (af on Sync, pT on the ACT HWDGE queue), so per cycle k the PE FIFO
    is [sum(k-1), att(k-1) x32 (data resident), scores(k) x16] and
    never waits on DMA; ACT FIFO is [pT(k+1) issues, tanh(k) x4,
    transpose(k), exp(k)] and never waits on late deps.

Engine budget/core per 13.8us cycle: DMA 5.25MiB (floor), PE ~11us,
ACT ~12us, DVE ~5us.

Layouts: region n = p*16 + c (att_feats/masks/scores); a = j*128 + p
(p_att/W_h).
"""

import numpy as np

B, N, H, A = 64, 2048, 1024, 512
NCORES = 8
BLOC = B // NCORES  # batches per core

P = 128
NT = N // P            # 16 n-columns per partition (att layout)
AJ = A // P            # 4 a-chunks (p_att layout)
NN = N // 512          # 4 score chunks of 512
HC = H // P            # 8 contraction chunks for the w_h matmul
AF_SUP = 4             # columns per att_feats supertile (4 DMAs per batch)

# p_att stream dtype: fp8e4m3 (device rel err 7.2e-3). Flip to bf16 if
# inputs change character.
PA_FP8 = True

_NC_CACHE = {}


def _build_nc():
    import concourse.bass as bass
    import concourse.mybir as mybir
    import concourse.tile as tile
    from concourse import bacc

    dt = mybir.dt
    f32, bf16 = dt.float32, dt.bfloat16
    pa_dt = dt.float8e4 if PA_FP8 else bf16
    AF = mybir.ActivationFunctionType
    OP = mybir.AluOpType

    nc = bacc.Bacc("TRN2", target_bir_lowering=False, debug=False,
                   num_devices=NCORES)

    paT = nc.dram_tensor("p_att_T", [BLOC, AJ, P, N], pa_dt,
                         kind="ExternalInput").ap()
    af = nc.dram_tensor("att_feats", [BLOC, N, H], bf16,
                        kind="ExternalInput").ap()
    whTj = nc.dram_tensor("W_hTj", [AJ, P, HC, P], bf16,
                          kind="ExternalInput").ap()
    hsT = nc.dram_tensor("hidden_T", [P, HC, BLOC], bf16,
                         kind="ExternalInput").ap()
    am = nc.dram_tensor("att_masks", [P, BLOC, NT], f32,
                        kind="ExternalInput").ap()
    waT = nc.dram_tensor("W_alphaT", [P, AJ], bf16, kind="ExternalInput").ap()
    out = nc.dram_tensor("att_out", [BLOC, H], f32, kind="ExternalOutput").ap()

    with tile.TileContext(nc) as tc:
        with (
            tc.tile_pool(name="consts", bufs=1) as consts,
            tc.tile_pool(name="patt", bufs=8) as patt_pool,
            tc.tile_pool(name="alpha", bufs=8) as alpha_pool,
            tc.tile_pool(name="afp", bufs=10) as af_pool,
            tc.tile_pool(name="small", bufs=3) as small,
            tc.tile_pool(name="arow", bufs=2) as arow_pool,
            tc.tile_pool(name="psc", bufs=4, space="PSUM") as psc_pool,
            tc.tile_pool(name="psatt", bufs=2, space="PSUM") as psatt,
            tc.tile_pool(name="psmisc", bufs=2, space="PSUM") as psmisc,
        ):
            af_r = [af[b, :, :].rearrange("(p c) h -> p c h", c=NT)
                    for b in range(BLOC)]

            # ---------------- prologue ----------------
            # Sync queue: consts then the first att_feats batch.
            whT_sb = []
            hidT_sb = consts.tile([P, HC, BLOC], bf16)
            for j in range(AJ):
                wt = consts.tile([P, HC, P], bf16, name=f"whT{j}")
                nc.sync.dma_start(out=wt, in_=whTj[j, :, :, :])
                whT_sb.append(wt)
                if j == 0:
                    nc.sync.dma_start(out=hidT_sb, in_=hsT)
            waT_sb = consts.tile([P, AJ], bf16)
            nc.sync.dma_start(out=waT_sb, in_=waT)
            masks_all = consts.tile([P, BLOC, NT], f32)
            nc.sync.dma_start(out=masks_all, in_=am)

            # ACT HWDGE queue: first p_att batch starts immediately.
            def dma_patt(b):
                tiles = []
                for j in range(AJ):
                    pt = patt_pool.tile([P, N], pa_dt, tag="patt",
                                        name=f"patt{b}_{j}")
                    nc.scalar.dma_start(out=pt, in_=paT[b, j, :, :])
                    tiles.append(pt)
                return tiles

            def dma_af(b):
                tiles = []
                for st in range(NT // AF_SUP):
                    aft = af_pool.tile([P, AF_SUP, H], bf16, tag="af",
                                       name=f"af{b}_{st}")
                    nc.sync.dma_start(
                        out=aft,
                        in_=af_r[b][:, st * AF_SUP:(st + 1) * AF_SUP, :])
                    tiles.append(aft)
                return tiles

            pt_tiles = {0: dma_patt(0)}
            af_tiles = {0: dma_af(0)}

            ones_col = consts.tile([P, 1], f32)
            nc.vector.memset(ones_col, 1.0)

            # w_hT[p, j, b] = sum_h W_h[j*128+p, h] * hidden[b, h]:
            # stationary W_h chunks put A on the OUTPUT partitions,
            # giving the transposed per-partition bias directly.
            whbias = consts.tile([P, AJ, BLOC], f32)
            for j in range(AJ):
                wh_ps = psc_pool.tile([P, BLOC], f32, tag="sc",
                                      name=f"whps{j}")
                for hc in range(HC):
                    nc.tensor.matmul(wh_ps, lhsT=whT_sb[j][:, hc, :],
                                     rhs=hidT_sb[:, hc, :],
                                     start=(hc == 0), stop=(hc == HC - 1))
                nc.scalar.copy(whbias[:, j, :], wh_ps)

            # ---------------- per-cycle phases ----------------
            def patt_compute(b):
                # fused add+tanh per a-chunk (data prefetched last cycle)
                alphas = []
                for j in range(AJ):
                    ab = alpha_pool.tile([P, N], bf16, tag="alpha",
                                         name=f"alpha{b}_{j}")
                    nc.scalar.activation(ab, pt_tiles[b][j], AF.Tanh,
                                         bias=whbias[:, j, b:b + 1])
                    alphas.append(ab)

                # scores on PE; row nn = scores for n in [512nn, 512nn+512)
                row4 = small.tile([1, N], f32, tag="row", name=f"row{b}")
                for nn in range(NN):
                    scp = psc_pool.tile([1, 512], f32, tag="sc",
                                        name=f"sc{b}_{nn}")
                    for j in range(AJ):
                        nc.tensor.matmul(
                            scp, lhsT=waT_sb[:, j:j + 1],
                            rhs=alphas[j][:, nn * 512:(nn + 1) * 512],
                            start=(j == 0), stop=(j == AJ - 1))
                    nc.vector.tensor_copy(row4[:, nn * 512:(nn + 1) * 512],
                                          scp)

                # tiny ACT-queue DMA: [1,2048] -> [128,16] (n = p*16+c)
                scT = small.tile([P, NT], f32, tag="scT", name=f"scT{b}")
                nc.scalar.dma_start(out=scT, in_=row4)

                nc.vector.tensor_tensor(out=scT, in0=scT,
                                        in1=masks_all[:, b, :], op=OP.add)
                expt = small.tile([P, NT], bf16, tag="expt", name=f"expt{b}")
                rowsum = small.tile([P, 1], f32, tag="rowsum",
                                    name=f"rowsum{b}")
                nc.scalar.activation(expt, scT, AF.Exp, accum_out=rowsum)
                return expt, rowsum

            def af_compute(b, expt, rowsum):
                sum_ps = psmisc.tile([1, 1], f32, tag="mm", name=f"sum{b}")
                nc.tensor.matmul(sum_ps, lhsT=rowsum, rhs=ones_col,
                                 start=True, stop=True)
                inv = small.tile([1, 1], f32, tag="inv", name=f"inv{b}")
                nc.vector.reciprocal(inv, sum_ps)

                att_lo = psatt.tile([1, A], f32, tag="att", name=f"attlo{b}")
                att_hi = psatt.tile([1, A], f32, tag="att", name=f"atthi{b}")
                tiles = af_tiles.pop(b)
                t = 0
                for st in range(NT // AF_SUP):
                    aft = tiles[st]
                    for c in range(AF_SUP):
                        lhs = expt[:, t:t + 1]
                        nc.tensor.matmul(att_lo, lhsT=lhs,
                                         rhs=aft[:, c, 0:A],
                                         start=(t == 0), stop=(t == NT - 1))
                        nc.tensor.matmul(att_hi, lhsT=lhs,
                                         rhs=aft[:, c, A:H],
                                         start=(t == 0), stop=(t == NT - 1))
                        t += 1

                # drain on the (otherwise idle) DVE with 1/sum folded in,
                # so the ACT queue never waits on late att matmuls
                att_row = arow_pool.tile([1, H], f32, tag="attrow",
                                         name=f"attrow{b}")
                nc.vector.tensor_scalar_mul(att_row[:, 0:A], att_lo, inv)
                nc.vector.tensor_scalar_mul(att_row[:, A:H], att_hi, inv)
                nc.gpsimd.dma_start(out=out[b:b + 1, :], in_=att_row)

            # ---------------- software pipeline ----------------
            state = {}
            for b in range(BLOC):
                if b >= 1:
                    af_compute(b - 1, *state.pop(b - 1))
                if b + 1 < BLOC:
                    pt_tiles[b + 1] = dma_patt(b + 1)
                    af_tiles[b + 1] = dma_af(b + 1)
                state[b] = patt_compute(b)
                pt_tiles.pop(b)
            af_compute(BLOC - 1, *state.pop(BLOC - 1))

    nc.compile()
    return nc


def _get_nc():
    if "nc" not in _NC_CACHE:
        _NC_CACHE["nc"] = _build_nc()
    return _NC_CACHE["nc"]


def kernel(hidden_states, att_feats, p_att_feats, att_masks, W_h, W_alpha):
    import ml_dtypes
    from concourse.bass_utils import run_bass_kernel_spmd

    nc = _get_nc()
    pa_np = ml_dtypes.float8_e4m3fn if PA_FP8 else ml_dtypes.bfloat16
    hidden_states = np.ascontiguousarray(hidden_states, dtype=np.float32)
    att_feats = np.ascontiguousarray(att_feats, dtype=np.float32)
    p_att_feats = np.ascontiguousarray(p_att_feats, dtype=np.float32)
    att_masks = np.ascontiguousarray(att_masks, dtype=np.float32)
    W_h = np.ascontiguousarray(W_h, dtype=np.float32)
    W_alpha = np.asarray(W_alpha, dtype=np.float32)

    # W_hTj[j, p, hc, m] = W_h[j*128+m, hc*128+p]
    whTj = np.ascontiguousarray(
        W_h.reshape(AJ, P, HC, P).transpose(0, 3, 2, 1)
    ).astype(ml_dtypes.bfloat16)
    waT = np.ascontiguousarray(
        W_alpha.reshape(AJ, P).T).astype(ml_dtypes.bfloat16)

    in_maps = []
    for i in range(NCORES):
        s = slice(i * BLOC, (i + 1) * BLOC)
        # [H, BLOC] -> [P, HC, BLOC]
        hT = np.ascontiguousarray(
            hidden_states[s].T.reshape(HC, P, BLOC).transpose(1, 0, 2)
        ).astype(ml_dtypes.bfloat16)
        # [BLOC, N] -> [P, BLOC, NT] with n = p*16 + c
        amr = np.ascontiguousarray(
            att_masks[s].reshape(BLOC, P, NT).transpose(1, 0, 2))
        # [BLOC, N, A] -> [BLOC, AJ, P, N] with a = j*128 + p
        paT = p_att_feats[s].transpose(0, 2, 1).reshape(
            BLOC, AJ, P, N).astype(pa_np)
        in_maps.append({
            "p_att_T": paT,
            "att_feats": att_feats[s].astype(ml_dtypes.bfloat16),
            "hidden_T": hT,
            "att_masks": amr,
            "W_hTj": whTj,
            "W_alphaT": waT,
        })

    global _LAST_IN_MAPS
    _LAST_IN_MAPS = in_maps
    res = run_bass_kernel_spmd(nc, in_maps, core_ids=list(range(NCORES)))
    return np.concatenate(
        [res.results[i]["att_out"] for i in range(NCORES)], axis=0
    ).astype(np.float32)


_LAST_IN_MAPS = None


# revision 7
# speedup vs baseline: 2.0814x; 1.1350x over previous
"""Bass/Tile TRN2 kernel for BasicAttention (v3: low-precision streams,
PE-based scores, full double-buffer prefetch).

att = softmax(tanh(hidden @ W_h.T + p_att_feats) @ W_alpha + mask) @ att_feats

Shapes: B=64, N=2048, H=1024, A=512. Data-parallel over batch across 8
NeuronCores (8 batches per core); weights replicated; no collectives.

Design (baseline all-f32 was 259-310us; v2 = 178us; v3 targets ~120us):
  * host casts the streams: att_feats -> bf16, p_att_feats -> fp8e4m3
    (device rel err 7.2e-3 vs 2e-2 budget). HBM: 102MB -> ~43MB/core.
  * p_att host-transposed to [b, j, a=128p, n]: A on partitions, so the
    w_h add is a per-partition scalar and add+tanh fuse into ONE ACT op
    (bias=w_hT column). No DVE add.
  * scores = alpha.T @ W_alpha on PE (lhsT=W_alphaT column, rhs=alpha
    chunk, 4 a-chunks accumulate into [1,512] PSUM rows).
  * score rows drain via DVE to a [1,2048] row; one tiny ACT-queue DMA
    transposes to region-partition layout [128,16] (n = p*16+c); mask
    add (DVE) + exp (ACT, accum rowsum) there; denominator via PE
    ones-matmul; att drains on DVE with 1/sum folded; SWDGE output.
  * both streams prefetch one full batch ahead on SEPARATE queues
    (af on Sync, pT on the ACT HWDGE queue), so per cycle k the PE FIFO
    is [sum(k-1), att(k-1) x32 (data resident), scores(k) x16] and
    never waits on DMA; ACT FIFO is [pT(k+1) issues, tanh(k) x4,
    transpose(k), exp(k)] and never waits on late deps.

Engine budget/core per 13.8us cycle: DMA 5.25MiB (floor), PE ~11us,
ACT ~12us, DVE ~5us.

Layouts: region n = p*16 + c (att_feats/masks/scores); a = j*128 + p
(p_att/W_h).
"""

import numpy as np

B, N, H, A = 64, 2048, 1024, 512
NCORES = 8
BLOC = B // NCORES  # batches per core

P = 128
NT = N // P            # 16 n-columns per partition (att layout)
AJ = A // P            # 4 a-chunks (p_att layout)
NN = N // 512          # 4 score chunks of 512
HC = H // P            # 8 contraction chunks for the w_h matmul
AF_SUP = 4             # columns per att_feats supertile (4 DMAs per batch)

# p_att stream dtype: fp8e4m3 (device rel err 7.2e-3). Flip to bf16 if
# inputs change character.
PA_FP8 = True

_NC_CACHE = {}


def _build_nc():
    import concourse.bass as bass
    import concourse.mybir as mybir
    import concourse.tile as tile
    from concourse import bacc

    dt = mybir.dt
    f32, bf16 = dt.float32, dt.bfloat16
    pa_dt = dt.float8e4 if PA_FP8 else bf16
    AF = mybir.ActivationFunctionType
    OP = mybir.AluOpType

    nc = bacc.Bacc("TRN2", target_bir_lowering=False, debug=False,
                   num_devices=NCORES)

    paT = nc.dram_tensor("p_att_T", [BLOC, AJ, P, N], pa_dt,
                         kind="ExternalInput").ap()
    af = nc.dram_tensor("att_feats", [BLOC, N, H], bf16,
                        kind="ExternalInput").ap()
    whTj = nc.dram_tensor("W_hTj", [AJ, P, HC, P], bf16,
                          kind="ExternalInput").ap()
    hsT = nc.dram_tensor("hidden_T", [P, HC, BLOC], bf16,
                         kind="ExternalInput").ap()
    am = nc.dram_tensor("att_masks", [P, BLOC, NT], f32,
                        kind="ExternalInput").ap()
    waT = nc.dram_tensor("W_alphaT", [P, AJ], bf16, kind="ExternalInput").ap()
    out = nc.dram_tensor("att_out", [BLOC, H], f32, kind="ExternalOutput").ap()

    with tile.TileContext(nc) as tc:
        with (
            tc.tile_pool(name="consts", bufs=1) as consts,
            tc.tile_pool(name="patt", bufs=8) as patt_pool,
            tc.tile_pool(name="alpha", bufs=6) as alpha_pool,
            tc.tile_pool(name="afp", bufs=12) as af_pool,
            tc.tile_pool(name="small", bufs=3) as small,
            tc.tile_pool(name="arow", bufs=2) as arow_pool,
            tc.tile_pool(name="psc", bufs=4, space="PSUM") as psc_pool,
            tc.tile_pool(name="psatt", bufs=2, space="PSUM") as psatt,
            tc.tile_pool(name="psmisc", bufs=2, space="PSUM") as psmisc,
        ):
            af_r = [af[b, :, :].rearrange("(p c) h -> p c h", c=NT)
                    for b in range(BLOC)]

            # ---------------- prologue ----------------
            # Sync queue: consts then the first att_feats batch.
            whT_sb = []
            hidT_sb = consts.tile([P, HC, BLOC], bf16)
            for j in range(AJ):
                wt = consts.tile([P, HC, P], bf16, name=f"whT{j}")
                nc.sync.dma_start(out=wt, in_=whTj[j, :, :, :])
                whT_sb.append(wt)
                if j == 0:
                    nc.sync.dma_start(out=hidT_sb, in_=hsT)
            waT_sb = consts.tile([P, AJ], bf16)
            nc.sync.dma_start(out=waT_sb, in_=waT)
            masks_all = consts.tile([P, BLOC, NT], f32)
            nc.sync.dma_start(out=masks_all, in_=am)

            # ACT HWDGE queue: first p_att batch starts immediately.
            def dma_patt(b):
                tiles = []
                for j in range(AJ):
                    pt = patt_pool.tile([P, N], pa_dt, tag="patt",
                                        name=f"patt{b}_{j}")
                    nc.scalar.dma_start(out=pt, in_=paT[b, j, :, :])
                    tiles.append(pt)
                return tiles

            def dma_af(b):
                tiles = []
                for st in range(NT // AF_SUP):
                    aft = af_pool.tile([P, AF_SUP, H], bf16, tag="af",
                                       name=f"af{b}_{st}")
                    nc.sync.dma_start(
                        out=aft,
                        in_=af_r[b][:, st * AF_SUP:(st + 1) * AF_SUP, :])
                    tiles.append(aft)
                return tiles

            pt_tiles = {0: dma_patt(0)}
            af_tiles = {0: dma_af(0)}

            ones_col = consts.tile([P, 1], f32)
            nc.vector.memset(ones_col, 1.0)

            # w_hT[p, j, b] = sum_h W_h[j*128+p, h] * hidden[b, h]:
            # stationary W_h chunks put A on the OUTPUT partitions,
            # giving the transposed per-partition bias directly.
            whbias = consts.tile([P, AJ, BLOC], f32)
            for j in range(AJ):
                wh_ps = psc_pool.tile([P, BLOC], f32, tag="sc",
                                      name=f"whps{j}")
                for hc in range(HC):
                    nc.tensor.matmul(wh_ps, lhsT=whT_sb[j][:, hc, :],
                                     rhs=hidT_sb[:, hc, :],
                                     start=(hc == 0), stop=(hc == HC - 1))
                nc.scalar.copy(whbias[:, j, :], wh_ps)

            # ---------------- per-cycle phases ----------------
            def patt_compute(b):
                # fused add+tanh per a-chunk (data prefetched last cycle)
                alphas = []
                for j in range(AJ):
                    ab = alpha_pool.tile([P, N], bf16, tag="alpha",
                                         name=f"alpha{b}_{j}")
                    nc.scalar.activation(ab, pt_tiles[b][j], AF.Tanh,
                                         bias=whbias[:, j, b:b + 1])
                    alphas.append(ab)

                # scores on PE; row nn = scores for n in [512nn, 512nn+512)
                row4 = small.tile([1, N], f32, tag="row", name=f"row{b}")
                for nn in range(NN):
                    scp = psc_pool.tile([1, 512], f32, tag="sc",
                                        name=f"sc{b}_{nn}")
                    for j in range(AJ):
                        nc.tensor.matmul(
                            scp, lhsT=waT_sb[:, j:j + 1],
                            rhs=alphas[j][:, nn * 512:(nn + 1) * 512],
                            start=(j == 0), stop=(j == AJ - 1))
                    nc.vector.tensor_copy(row4[:, nn * 512:(nn + 1) * 512],
                                          scp)

                # tiny ACT-queue DMA: [1,2048] -> [128,16] (n = p*16+c)
                scT = small.tile([P, NT], f32, tag="scT", name=f"scT{b}")
                nc.scalar.dma_start(out=scT, in_=row4)

                nc.vector.tensor_tensor(out=scT, in0=scT,
                                        in1=masks_all[:, b, :], op=OP.add)
                expt = small.tile([P, NT], bf16, tag="expt", name=f"expt{b}")
                rowsum = small.tile([P, 1], f32, tag="rowsum",
                                    name=f"rowsum{b}")
                nc.scalar.activation(expt, scT, AF.Exp, accum_out=rowsum)
                return expt, rowsum

            def sum_phase(b, rowsum):
                # cycle b+1: softmax denominator; inv(b) is ready a full
                # cycle before the att drains need it
                sum_ps = psmisc.tile([1, 1], f32, tag="mm", name=f"sum{b}")
                nc.tensor.matmul(sum_ps, lhsT=rowsum, rhs=ones_col,
                                 start=True, stop=True)
                inv = small.tile([1, 1], f32, tag="inv", name=f"inv{b}")
                nc.vector.reciprocal(inv, sum_ps)
                return inv

            def att_phase(b, expt, inv):
                # cycle b+2: every dependency (expt, inv, af tiles) is
                # >=1 cycle old, so the PE never waits here
                att_lo = psatt.tile([1, A], f32, tag="att", name=f"attlo{b}")
                att_hi = psatt.tile([1, A], f32, tag="att", name=f"atthi{b}")
                tiles = af_tiles.pop(b)  # loaded two cycles ago
                t = 0
                for st in range(NT // AF_SUP):
                    aft = tiles[st]
                    for c in range(AF_SUP):
                        lhs = expt[:, t:t + 1]
                        nc.tensor.matmul(att_lo, lhsT=lhs,
                                         rhs=aft[:, c, 0:A],
                                         start=(t == 0), stop=(t == NT - 1))
                        nc.tensor.matmul(att_hi, lhsT=lhs,
                                         rhs=aft[:, c, A:H],
                                         start=(t == 0), stop=(t == NT - 1))
                        t += 1

                # drain on the (otherwise idle) DVE with 1/sum folded in,
                # so the ACT queue never waits on late att matmuls
                att_row = arow_pool.tile([1, H], f32, tag="attrow",
                                         name=f"attrow{b}")
                nc.vector.tensor_scalar_mul(att_row[:, 0:A], att_lo, inv)
                nc.vector.tensor_scalar_mul(att_row[:, A:H], att_hi, inv)
                nc.gpsimd.dma_start(out=out[b:b + 1, :], in_=att_row)

            # ---------------- software pipeline (depth 2) ----------------
            # cycle k: att(k-2) | sum(k-1) | stream af(k)/pT(k+1) | tanh/
            # scores/exp(k).  The serial score->expt chain of batch k
            # overlaps the att matmuls of batch k-2 on the PE.
            expt_d, rowsum_d, inv_d = {}, {}, {}
            for k in range(BLOC + 2):
                if k >= 2:
                    att_phase(k - 2, expt_d.pop(k - 2), inv_d.pop(k - 2))
                if 1 <= k <= BLOC:
                    inv_d[k - 1] = sum_phase(k - 1, rowsum_d.pop(k - 1))
                if k + 1 < BLOC:
                    pt_tiles[k + 1] = dma_patt(k + 1)
                if k < BLOC:
                    if k >= 1:
                        af_tiles[k] = dma_af(k)
                    expt_d[k], rowsum_d[k] = patt_compute(k)
                    pt_tiles.pop(k)

    nc.compile()
    return nc


def _get_nc():
    if "nc" not in _NC_CACHE:
        _NC_CACHE["nc"] = _build_nc()
    return _NC_CACHE["nc"]


def kernel(hidden_states, att_feats, p_att_feats, att_masks, W_h, W_alpha):
    import ml_dtypes
    from concourse.bass_utils import run_bass_kernel_spmd

    nc = _get_nc()
    pa_np = ml_dtypes.float8_e4m3fn if PA_FP8 else ml_dtypes.bfloat16
    hidden_states = np.ascontiguousarray(hidden_states, dtype=np.float32)
    att_feats = np.ascontiguousarray(att_feats, dtype=np.float32)
    p_att_feats = np.ascontiguousarray(p_att_feats, dtype=np.float32)
    att_masks = np.ascontiguousarray(att_masks, dtype=np.float32)
    W_h = np.ascontiguousarray(W_h, dtype=np.float32)
    W_alpha = np.asarray(W_alpha, dtype=np.float32)

    # W_hTj[j, p, hc, m] = W_h[j*128+m, hc*128+p]
    whTj = np.ascontiguousarray(
        W_h.reshape(AJ, P, HC, P).transpose(0, 3, 2, 1)
    ).astype(ml_dtypes.bfloat16)
    waT = np.ascontiguousarray(
        W_alpha.reshape(AJ, P).T).astype(ml_dtypes.bfloat16)

    in_maps = []
    for i in range(NCORES):
        s = slice(i * BLOC, (i + 1) * BLOC)
        # [H, BLOC] -> [P, HC, BLOC]
        hT = np.ascontiguousarray(
            hidden_states[s].T.reshape(HC, P, BLOC).transpose(1, 0, 2)
        ).astype(ml_dtypes.bfloat16)
        # [BLOC, N] -> [P, BLOC, NT] with n = p*16 + c
        amr = np.ascontiguousarray(
            att_masks[s].reshape(BLOC, P, NT).transpose(1, 0, 2))
        # [BLOC, N, A] -> [BLOC, AJ, P, N] with a = j*128 + p
        paT = p_att_feats[s].transpose(0, 2, 1).reshape(
            BLOC, AJ, P, N).astype(pa_np)
        in_maps.append({
            "p_att_T": paT,
            "att_feats": att_feats[s].astype(ml_dtypes.bfloat16),
            "hidden_T": hT,
            "att_masks": amr,
            "W_hTj": whTj,
            "W_alphaT": waT,
        })

    global _LAST_IN_MAPS
    _LAST_IN_MAPS = in_maps
    res = run_bass_kernel_spmd(nc, in_maps, core_ids=list(range(NCORES)))
    return np.concatenate(
        [res.results[i]["att_out"] for i in range(NCORES)], axis=0
    ).astype(np.float32)


_LAST_IN_MAPS = None
